# revision 6
# baseline (speedup 1.0000x reference)
"""Trainium2 Bass kernel for nn_LossFunc_69372311765146 (moe_routing).

Computation (only the last of the 11 unrolled states survives in the
reference, so the heavy work reduces to per-row softmax statistics of
logits [262144, 1000]):
    logp_k = logits[r, t_r] - logsumexp(logits[r, :])
    p_k    = exp(logp_k)
    p_j    = max prob strictly below p_k   (only if routing selects it)
    res    = BRANCH[idx](x1, x2),  x* in {p_k, p_j, 1}
    loss   = sum(-(w*res)**5 * logp_k)

Device kernel (data parallel over 8 cores, 32768 rows each, tiles of
128 rows x 1000 cols):
    ScalarE: e = exp(l) with accum_out -> Z row-sums
    VectorE: (iota == target) * l with accum_out(sum) -> l_k gather
    [p_j path only] VectorE: (l < l_k) * e -> masked e, reduce_max -> e_j
Host: tiny routing math, final per-row loss assembly + all-reduce (f64).
"""

import numpy as np

N, C = 262144, 1000
NCORES = 8
R = N // NCORES        # 32768 rows per core
P = 128                # partitions
TILES = R // P         # 256 tiles per core
BLK = 2                # row-blocks per DMA (1 MB per dma_start)
TAU = 0.1
GAMMA = 5
EPS = 1e-12


def _build_bass(need_pj: bool, rows: int = R, cols: int = C):
    import concourse.bacc as bacc
    import concourse.mybir as mybir
    import concourse.tile as tile

    tiles = rows // P
    blk = BLK if tiles % BLK == 0 else 1
    F32 = mybir.dt.float32
    Alu = mybir.AluOpType
    Act = mybir.ActivationFunctionType
    Ax = mybir.AxisListType

    nc = bacc.Bacc("TRN2", target_bir_lowering=False, debug=False)
    logits = nc.dram_tensor("logits", [rows, cols], F32, kind="ExternalInput").ap()
    tcols = nc.dram_tensor("tcols", [P, tiles], F32, kind="ExternalInput").ap()
    iota = nc.dram_tensor("iota", [P, cols], F32, kind="ExternalInput").ap()
    z_out = nc.dram_tensor("z_out", [P, tiles], F32, kind="ExternalOutput").ap()
    lk_out = nc.dram_tensor("lk_out", [P, tiles], F32, kind="ExternalOutput").ap()
    ej_out = None
    if need_pj:
        ej_out = nc.dram_tensor("ej_out", [P, tiles], F32, kind="ExternalOutput").ap()

    lr = logits.rearrange("(n p) c -> p n c", p=P)  # tile n on partition p = row n*P+p

    with tile.TileContext(nc) as tc:
        with tc.tile_pool(name="lp", bufs=4) as lp, \
             tc.tile_pool(name="ep", bufs=3) as ep, \
             tc.tile_pool(name="jp", bufs=3) as jp, \
             tc.tile_pool(name="cp", bufs=1) as cp, \
             tc.tile_pool(name="sp", bufs=1) as sp:
            iota_t = cp.tile([P, cols], F32, tag="iota")
            nc.sync.dma_start(out=iota_t[:], in_=iota)
            tcols_t = cp.tile([P, tiles], F32, tag="tcols")
            nc.sync.dma_start(out=tcols_t[:], in_=tcols)
            z_sb = sp.tile([P, tiles], F32, tag="z")
            lk_sb = sp.tile([P, tiles], F32, tag="lk")
            ej_sb = None
            if need_pj:
                ej_sb = sp.tile([P, tiles], F32, tag="ej")

            for d in range(tiles // blk):
                lt = lp.tile([P, blk, cols], F32, tag="l")
                nc.sync.dma_start(out=lt[:], in_=lr[:, d * blk:(d + 1) * blk, :])
                for j in range(blk):
                    i = d * blk + j
                    et = ep.tile([P, cols], F32, tag="e")
                    nc.scalar.activation(
                        et[:], lt[:, j, :], Act.Exp, accum_out=z_sb[:, i:i + 1]
                    )
                    jt = jp.tile([P, cols], F32, tag="j")
                    nc.vector.scalar_tensor_tensor(
                        out=jt[:], in0=iota_t[:], scalar=tcols_t[:, i:i + 1],
                        in1=lt[:, j, :], op0=Alu.is_equal, op1=Alu.mult,
                        accum_out=lk_sb[:, i:i + 1],
                    )
                    if need_pj:
                        mt = jp.tile([P, cols], F32, tag="m")
                        nc.vector.scalar_tensor_tensor(
                            out=mt[:], in0=lt[:, j, :], scalar=lk_sb[:, i:i + 1],
                            in1=et[:], op0=Alu.is_lt, op1=Alu.mult,
                        )
                        nc.vector.tensor_reduce(
                            out=ej_sb[:, i:i + 1], in_=mt[:], axis=Ax.X, op=Alu.max
                        )
            nc.sync.dma_start(out=z_out, in_=z_sb[:])
            nc.sync.dma_start(out=lk_out, in_=lk_sb[:])
            if need_pj:
                nc.sync.dma_start(out=ej_out, in_=ej_sb[:])
    nc.compile()
    return nc


def _routing(alphas_ops, alphas_operators, g_ops, g_operators):
    """Replicate the reference's gumbel-softmax routing for state 10."""
    s_ops = (np.asarray(alphas_ops, np.float32) + np.asarray(g_ops, np.float32)) / TAU
    s_opr = (np.asarray(alphas_operators, np.float32)
             + np.asarray(g_operators, np.float32)) / TAU
    i = 10
    idx = int(np.argmax(s_ops[i]))
    # softmax value for the winning weight (f32, like jax.nn.softmax)
    e = np.exp(s_ops[i] - s_ops[i].max())
    w = float(e[idx] / e.sum())
    top2 = np.argsort(-s_opr[i], kind="stable")[:2]
    names = ["p_k", "p_j", "ones", "p_k", "p_j", "ones", "p_k", "p_j"]
    x1, x2 = names[int(top2[0])], names[int(top2[1])]
    return idx, w, x1, x2


def _branch(idx, a, b):
    if idx == 0:
        return a + b
    if idx == 1:
        return a * b
    if idx == 2:
        return a - b
    if idx == 3:
        return a / (b + EPS)
    if idx == 4:
        return np.maximum(a, b)
    if idx == 5:
        return np.minimum(a, b)
    if idx == 6:
        return a * (1.0 / (1.0 + np.exp(-b)))
    if idx == 7:
        return np.abs(a - b)
    raise ValueError(idx)


def _prepare(logits, target, alphas_ops, alphas_operators, g_ops, g_operators):
    logits = np.ascontiguousarray(np.asarray(logits, dtype=np.float32))
    target = np.asarray(target)
    assert logits.shape == (N, C), logits.shape

    idx, w, x1, x2 = _routing(alphas_ops, alphas_operators, g_ops, g_operators)
    # p_j is strictly below p_k (and p_k <= 1), so under `maximum` it never
    # wins against p_k or ones -> substituting 0 for p_j is exact there.
    need_pj = "p_j" in (x1, x2) and not (
        idx == 4 and (x1, x2) != ("p_j", "p_j")
    )

    nc = _build_bass(need_pj)

    iota = np.tile(np.arange(C, dtype=np.float32), (P, 1))
    in_maps = []
    for c in range(NCORES):
        lsh = logits[c * R:(c + 1) * R]
        tsh = target[c * R:(c + 1) * R]
        tcols = np.ascontiguousarray(
            tsh.reshape(TILES, P).T.astype(np.float32)
        )
        in_maps.append({"logits": lsh, "tcols": tcols, "iota": iota})
    meta = (idx, w, x1, x2, need_pj)
    return nc, in_maps, meta


def _finalize(outs, meta):
    idx, w, x1, x2, need_pj = meta
    z = np.concatenate(
        [o["z_out"].T.reshape(-1) for o in outs]).astype(np.float64)
    lk = np.concatenate(
        [o["lk_out"].T.reshape(-1) for o in outs]).astype(np.float64)
    logp_k = lk - np.log(z)
    vals = {"p_k": np.exp(logp_k), "ones": 1.0, "p_j": 0.0}
    if need_pj:
        ej = np.concatenate(
            [o["ej_out"].T.reshape(-1) for o in outs]).astype(np.float64)
        vals["p_j"] = ej / z
    last = w * _branch(idx, vals[x1], vals[x2])
    loss = np.sum(-(last ** GAMMA) * logp_k)
    return np.array(loss, dtype=np.float32)


def kernel(logits, target, alphas_ops, alphas_operators, g_ops, g_operators):
    from concourse.bass_utils import run_bass_kernel_spmd

    nc, in_maps, meta = _prepare(
        logits, target, alphas_ops, alphas_operators, g_ops, g_operators)
    res = run_bass_kernel_spmd(nc, in_maps, core_ids=list(range(NCORES)))
    globals()["LAST_RESULTS"] = res
    return _finalize(res.results, meta)
